# revision 7
# baseline (speedup 1.0000x reference)
"""Trainium2 Bass kernel for nn_Attention_7421703487529.

Multi-head attention, B=4 N=2048 C=512 H=8 D=64, fp32.
Sharding: 8 cores = 4 batches x 2 head-groups (4 heads each). No collectives.

Design (all per core; 128 iterations over (chunk c:4, pair p:2, key-tile i:16)):
- The wall is exp throughput: 4 heads x 2048^2 softmax exps. Split across TWO
  engines: ScalarE runs [128,1024] exp ACTIVATEs (~1.11us each, covers both
  heads of a pair for a 512-query chunk); DVE runs Schraudolph exp for tiles
  with i%2==0, i>0, m>=16 (49 of 128): one tensor_scalar P_int =
  int32(s*(2^23*log2 e*SCALE) + B) whose int32 bits ARE the fp32 exp approx;
  the AV matmul reads the high bf16 halves via a stride-2 AP. B is centered
  (mean multiplicative bias ~1) and +0x8000 rounds-to-nearest the bf16 half.
- AV-deferral: for DVE-offloaded tiles the AV pair is deferred one iteration
  (emitted after the NEXT S-pair) so a late DVE tensor_scalar can never stall
  the PE before S(k+3) is out -> the ScalarE exp stream never starves while
  skipping offloaded tiles.
- Score matmuls (K=64) for the two heads run CONCURRENTLY via PE row-tiling
  into one [128,1024] PSUM tile; S(k+2) is emitted before AV(k).
- V is computed directly in [token, dim] layout into 66-wide slots with a
  ones column at index 64 -> the AV matmul accumulates numerator rows 0-63 +
  denominator row 64.
- Steady normalize: aug->SBUF copy (DVE), denominator spread via SBUF-SBUF DMA
  (sync queue), 16-wide reciprocal (DVE), DRAM-bounce broadcast (gpsimd
  queue), final mul on the GPSIMD engine (all-SBUF operands). The bc+mul is
  deferred via the filler queue so a late broadcast can't stall the PE.
- Projection: psum + bias staged as ONE [128,2048] f16 tile per chunk via DVE
  tensor_scalar add (bias as per-partition scalar AP), stored with a single
  128-descriptor DMA on the sync queue (4x fewer store descriptors).
- Tail (last pair): DMA-free normalize chain with PE-keepalive K=1 matmuls
  data-dep-pinned to the chain (a >3.4us PE stall trips HAM re-throttle to
  1.2 GHz); projections' bias-adds on the then-idle ScalarE; final store as
  2 half DMAs on sync+scalar queues.
- Inputs land as big DMAs from host-interleaved [128, x] layouts; ~104 tiny
  warm-up matmuls bridge the DMA wait to keep the PE HAM clock at 2.4 GHz.
"""

import sys

for _p in ("/opt/trn_rl_repo", "/root/.axon_site/_ro/trn_rl_repo"):
    if _p not in sys.path:
        sys.path.append(_p)

import ml_dtypes
import numpy as np

import concourse.bass as bass
import concourse.tile as tile
from concourse import bacc, mybir
from concourse.bass_utils import run_bass_kernel_spmd

F32 = mybir.dt.float32
F16 = mybir.dt.float16
BF16 = mybir.dt.bfloat16

B, N, C = 4, 2048, 512
H, D = 8, 64
HG = 2              # head-groups (cores per batch)
HL = H // HG        # heads per core (4)
CG = C // HG        # channels per group (256)
SCALE = D ** -0.5
P = 128             # partitions
NT = N // P         # 16 key tiles per head
NCH = 4             # query chunks
QC = N // NCH       # 512 queries per chunk
SLOT = 66           # vall slot width: 64 V dims + ones col (64) + pad
EXP = mybir.ActivationFunctionType.Exp


def _build_body(nc, xT, wqk, wv, wpT, bias, yT):
    from contextlib import ExitStack

    with tile.TileContext(nc) as tc, ExitStack() as ctx:
        consts = ctx.enter_context(tc.tile_pool(name="consts", bufs=1))
        pT_pool = ctx.enter_context(tc.tile_pool(name="pT", bufs=6))
        cpa_pool = ctx.enter_context(tc.tile_pool(name="cpa", bufs=3))
        rec_pool = ctx.enter_context(tc.tile_pool(name="rec", bufs=3))
        stg_pool = ctx.enter_context(tc.tile_pool(name="stg", bufs=2))
        pS_pool = ctx.enter_context(tc.tile_pool(name="pS", bufs=3))
        dram_pool = ctx.enter_context(tc.tile_pool(name="drp", bufs=2, space="DRAM"))
        # per-head score tiles: releasing ssA (after the head-A half-exp or
        # half-Schraudolph) unblocks the next S_A matmul while head B is
        # still being consumed -> the ss-buf recycle chain (exp(k-1) ->
        # S(k+1) -> exp(k+1)) hides behind the half-exp instead of opening
        # a ~0.6us ScalarE hole at every offloaded tile
        ssA_pool = ctx.enter_context(tc.tile_pool(name="ssA", bufs=2, space="PSUM"))
        ssB_pool = ctx.enter_context(tc.tile_pool(name="ssB", bufs=2, space="PSUM"))
        aug_pool = ctx.enter_context(tc.tile_pool(name="aug", bufs=2, space="PSUM"))
        fil_pool = ctx.enter_context(tc.tile_pool(name="fil", bufs=2, space="PSUM"))

        # ---- HAM warm-up: tiny back-to-back matmuls promote the PE clock to
        # 2.4 GHz while input DMAs are in flight (no DMA dependency) ----
        wsrc = consts.tile([64, 64], BF16, tag="wsrc")
        nc.vector.memset(wsrc.bitcast(mybir.dt.uint16), 0x3F80)
        warm = fil_pool.tile([64, 64], F32, tag="fil", name="warm")
        for _ in range(104):
            nc.tensor.matmul(warm[:, :], lhsT=wsrc[:, :], rhs=wsrc[:, :],
                             start=True, stop=True)

        # ---- input loads: few big DMAs from host-interleaved layouts ----
        # xT dram: [128, chunk q (4) x ct (4) x 512]; wqk: [128, ct x 512]
        # (ct-blocks each [Q01|K01|Q23|K23]); wv: [128, ct x 256]; wp:
        # [128, ct2 x 512]
        xT_all = consts.tile([P, 4 * N], BF16, tag="xT_all")
        wqk_all = consts.tile([P, 4 * 512], BF16, tag="wqk_all")
        wv_all = consts.tile([P, 4 * CG], BF16, tag="wv_all")
        wp_all = consts.tile([P, 2 * C], BF16, tag="wp_all")
        bias_sb = consts.tile([P, 4], F32, tag="bias")

        nc.sync.dma_start(out=wqk_all[:, 0:512], in_=wqk[:, 0:512])
        nc.gpsimd.dma_start(out=xT_all[:, 0:512], in_=xT[:, 0:512])
        nc.scalar.dma_start(out=xT_all[:, 1024:1536], in_=xT[:, 1024:1536])
        nc.sync.dma_start(out=wqk_all[:, 512:1024], in_=wqk[:, 512:1024])
        nc.gpsimd.dma_start(out=xT_all[:, 512:1024], in_=xT[:, 512:1024])
        nc.scalar.dma_start(out=xT_all[:, 1536:2048], in_=xT[:, 1536:2048])
        nc.sync.dma_start(out=xT_all[:, 2048:4096], in_=xT[:, 2048:4096])
        nc.gpsimd.dma_start(out=xT_all[:, 4096:6144], in_=xT[:, 4096:6144])
        nc.scalar.dma_start(out=wv_all, in_=wv[:, :])
        nc.sync.dma_start(out=wqk_all[:, 1024:2048], in_=wqk[:, 1024:2048])
        nc.gpsimd.dma_start(out=xT_all[:, 6144:8192], in_=xT[:, 6144:8192])
        nc.scalar.dma_start(out=wp_all, in_=wpT[:, :])
        nc.scalar.dma_start(
            out=bias_sb,
            in_=bass.AP(tensor=bias.tensor, offset=bias.offset, ap=[[1, P], [P, 4]]),
        )
        # trigger the exp ACT-table load (~2.7us) before the first real exp;
        # placed after the scalar-queue DMA issues so those aren't delayed
        actwarm = consts.tile([1, 1], F32, tag="actwarm")
        nc.scalar.activation(out=actwarm[0:1, :],
                             in_=nc.const_aps.aps[(F32, 1.0)][0:1, :],
                             func=EXP, scale=1.0)

        def x_sl(ct, q, j0, w):
            return xT_all[:, q * 2048 + ct * QC + j0: q * 2048 + ct * QC + j0 + w]

        # ---- SBUF working tensors ----
        # qkvT tiles: 0=Q01, 1=K01, 2=Q23, 3=K23 ([128, N]: pair head A rows
        # 0-63, head B rows 64-127)
        qkvT_sb = [consts.tile([P, N], BF16, tag=f"qkvT{jt}", name=f"qkvT{jt}")
                   for jt in range(4)]
        # V slots: [p, i, l, c]: c=0:64 V dims, c=64 ones, c=65 pad
        vall = consts.tile([P, NT * HL * SLOT], BF16, tag="vall")
        vall4 = vall.rearrange("p (i l c) -> p i l c", l=HL, c=SLOT)
        nc.vector.memset(vall4[:, :, :, 64].bitcast(mybir.dt.uint16), 0x3F80)
        nc.vector.memset(vall4[:, :, :, 65].bitcast(mybir.dt.uint16), 0)
        outT_sb = [consts.tile([P, N], BF16, tag=f"outT{t}", name=f"outT{t}")
                   for t in range(2)]
        ones_col = consts.tile([P, 64], BF16, tag="ones_col")
        nc.vector.memset(ones_col.bitcast(mybir.dt.uint16), 0x3F80)

        # ---- emission helpers ----
        def qkv_quarter(jt, q):
            # qkvT_sb[jt][:, q*512:(q+1)*512] = wqk[:, jt].T @ x[:, qchunk]
            ps = fil_pool.tile([P, QC], F32, tag="fil", name="ps_qkv")
            for ct in range(4):
                nc.tensor.matmul(
                    ps[:, :],
                    lhsT=wqk_all[:, jt * 512 + ct * P:jt * 512 + (ct + 1) * P],
                    rhs=x_sl(ct, q, 0, QC),
                    start=(ct == 0),
                    stop=(ct == 3),
                )
            nc.vector.tensor_copy(out=qkvT_sb[jt][:, q * QC:(q + 1) * QC],
                                  in_=ps[:, :])

        def v_unit(i):
            # V[tokens 128i.., dims 256] = x_chunk @ wv ; scatter to head slots
            ps = fil_pool.tile([P, CG], F32, tag="fil", name="ps_v")
            for ct in range(4):
                nc.tensor.matmul(
                    ps[:, :],
                    lhsT=x_sl(ct, i // 4, (i % 4) * P, P),
                    rhs=wv_all[:, ct * CG:(ct + 1) * CG],
                    start=(ct == 0),
                    stop=(ct == 3),
                )
            ps3 = ps.rearrange("p (l c) -> p l c", l=HL)
            nc.vector.tensor_copy(out=vall4[:, i, :, 0:64], in_=ps3[:, :, :])

        # f16 output staging: one [128, 4*QC] tile per chunk; col ot*QC+j
        stages = {}

        def proj_unit(ot, c, tail=False):
            # y[ot rows, chunk c] = wp.T-slices @ outT + bias -> f16 staging
            ps = fil_pool.tile([P, QC], F32, tag="fil", name="ps_y")
            for ct in range(2):
                nc.tensor.matmul(
                    ps[:, :],
                    lhsT=wp_all[:, ct * C + ot * P:ct * C + (ot + 1) * P],
                    rhs=outT_sb[ct][:, c * QC:(c + 1) * QC],
                    start=(ct == 0),
                    stop=(ct == 1),
                )
            if ot == 0:
                stages[c] = stg_pool.tile([P, 4 * QC], F16, tag="stg",
                                          name=f"stg{c}")
            stg = stages[c]
            if tail:
                nc.scalar.add(out=stg[:, ot * QC:(ot + 1) * QC], in_=ps[:, :],
                              add=bias_sb[:, ot:ot + 1])
            else:
                nc.vector.tensor_scalar(
                    out=stg[:, ot * QC:(ot + 1) * QC], in0=ps[:, :],
                    scalar1=bias_sb[:, ot:ot + 1], scalar2=None,
                    op0=mybir.AluOpType.add)
            if ot == 3:
                if tail:
                    nc.sync.dma_start(out=yT[:, c * 2048:c * 2048 + 1024],
                                      in_=stg[:, 0:1024])
                    nc.scalar.dma_start(out=yT[:, c * 2048 + 1024:(c + 1) * 2048],
                                        in_=stg[:, 1024:2048])
                else:
                    nc.sync.dma_start(out=yT[:, c * 2048:(c + 1) * 2048],
                                      in_=stg[:, :])

        # ---- prologue compute: ONLY what exp#0 needs (cold clock) ----
        qkv_quarter(0, 0)      # Q01 cols 0:512
        qkv_quarter(1, 0)      # K01 cols 0:512 (key tiles 0-3)

        # filler queue: (deadline iteration, fn). 128 iterations total;
        # iteration index = 32*c + 16*p + i. qkv quarters are due >=4 iters
        # before first use so their DVE copies clear the boundary DVE bursts.
        fillers = []

        def q(dl, fn, *a):
            fillers.append((dl, lambda: fn(*a)))

        for i in range(NT):
            q(i - 1, v_unit, i)                     # V token tiles (JIT)
        q(0, qkv_quarter, 1, 1)                     # K01 tiles 4-7
        q(4, qkv_quarter, 1, 2)
        q(8, qkv_quarter, 1, 3)
        q(11, qkv_quarter, 2, 0)                    # Q23 cols 0:512
        q(12, qkv_quarter, 3, 0)                    # K23 tiles 0-3
        q(14, qkv_quarter, 3, 1)
        q(16, qkv_quarter, 3, 2)
        q(20, qkv_quarter, 3, 3)
        q(24, qkv_quarter, 0, 1)                    # Q01 chunk 1
        q(40, qkv_quarter, 2, 1)                    # Q23 chunk 1
        q(56, qkv_quarter, 0, 2)
        q(72, qkv_quarter, 2, 2)
        q(88, qkv_quarter, 0, 3)
        q(104, qkv_quarter, 2, 3)
        fillers.sort(key=lambda u: u[0])

        # ---- attention: 128 iterations of (chunk, pair, key-tile) ----
        blocks = [(c, p) for c in range(NCH) for p in range(2)]
        iters = [(c, p, i) for (c, p) in blocks for i in range(NT)]

        def emit_S(k):
            c, p, i = iters[k]
            QT, KT = qkvT_sb[2 * p], qkvT_sb[2 * p + 1]
            ssA = ssA_pool.tile([P, QC], F32, tag="ssA", name="ssA")
            ssB = ssB_pool.tile([P, QC], F32, tag="ssB", name="ssB")
            nc.tensor.matmul(
                ssA[:, :],
                lhsT=KT[0:64, i * P:(i + 1) * P],
                rhs=QT[0:64, c * QC:(c + 1) * QC],
                start=True, stop=True,
            )
            nc.tensor.matmul(
                ssB[:, :],
                lhsT=KT[64:128, i * P:(i + 1) * P],
                rhs=QT[64:128, c * QC:(c + 1) * QC],
                start=True, stop=True,
            )
            return (ssA, ssB)

        # Schraudolph exp on DVE for offloaded tiles: P = bitcast(int32(
        # s*A + B)); +0x8000 rounds-to-nearest the bf16 high half the AV
        # matmul reads via a stride-2 AP. Any per-tile scalar bias cancels
        # between softmax numerator and denominator.
        SCHRAU_A = float(SCALE * (2 ** 23) / np.log(2.0))
        SCHRAU_B = float(127 * 2 ** 23 - 482870 + 0x8000)
        I32 = mybir.dt.int32

        def offloaded(m):
            ci, pi, ii = iters[m]
            return ii % 2 == 0 and ii > 0 and m >= 16

        def emit_tile(m):
            # S-pair matmuls; for DVE-offloaded tiles the Schraudolph
            # tensor_scalars are emitted HERE (2 iterations ahead in the DVE
            # queue) so they start as soon as the scores land; per-head ops
            # release ssA early for the next S_A
            ssab = emit_S(m)
            if offloaded(m):
                pS = pS_pool.tile([P, 2 * QC], I32, tag="pS", name="pS")
                for h in range(2):
                    nc.vector.tensor_scalar(out=pS[:, h * QC:(h + 1) * QC],
                                            in0=ssab[h][:, :],
                                            scalar1=SCHRAU_A, scalar2=SCHRAU_B,
                                            op0=mybir.AluOpType.mult,
                                            op1=mybir.AluOpType.add)
                return (ssab, [
                    pS[:, h * QC:(h + 1) * QC].bitcast(BF16)
                    .rearrange("p (n two) -> p n two", two=2)[:, :, 1]
                    for h in range(2)
                ])
            return (ssab, None)

        def emit_AV(augs_, p_, i_, rhs_):
            for h01 in range(2):
                nc.tensor.matmul(
                    augs_[h01][:, :],
                    lhsT=vall4[:, i_, 2 * p_ + h01, 0:65],
                    rhs=rhs_[h01],
                    start=(i_ == 0),
                    stop=(i_ == NT - 1),
                )

        ss_q = [emit_tile(0), emit_tile(1)]
        augs = None
        deferred = None      # (augs, p, i, rhs_ab) AV pair deferred one iter
        done = 0
        for k, (c, p, i) in enumerate(iters):
            if i == 0:
                augs = (aug_pool.tile([65, QC], F32, tag="aug", name="augA"),
                        aug_pool.tile([65, QC], F32, tag="aug", name="augB"))
            ssab, pre_rhs = ss_q.pop(0)
            if pre_rhs is not None:
                rhs_ab = pre_rhs
            else:
                pT = pT_pool.tile([P, 2 * QC], BF16, tag="pT")
                for h in range(2):
                    nc.scalar.activation(out=pT[:, h * QC:(h + 1) * QC],
                                         in_=ssab[h][:, :], func=EXP,
                                         scale=float(SCALE))
                rhs_ab = [pT[:, 0:QC], pT[:, QC:2 * QC]]
            # flush the PREVIOUS iteration's deferred AV pair FIRST: its
            # inputs are a full slot old -> wait-free, keeps the PE pipeline
            # back-to-back and hides the ss-recycle chain (the next S-pair's
            # psum-buf wait on this slot's half-exp) behind real work
            if deferred is not None:
                emit_AV(*deferred)
                deferred = None
            if k + 2 < len(iters):
                ss_q.append(emit_tile(k + 2))
            # fillers: force-pop overdue units (correctness: a unit's write
            # must be EMITTED before its first reader), then a bounded
            # near-deadline drain
            npop = 0
            while fillers and fillers[0][0] <= done:
                fillers.pop(0)[1]()
                npop += 1
            while fillers and npop < 2 and (fillers[0][0] <= done + 3
                                            or (done % 3 == 0 and npop == 0)):
                fillers.pop(0)[1]()
                npop += 1
            done += 1

            if i < NT - 1:
                # defer EVERY AV pair one iteration (except i=15: the aug
                # pool has no room to let accumulation cross a pair boundary)
                deferred = (augs, p, i, rhs_ab)
            else:
                emit_AV(augs, p, i, rhs_ab)
            if i == NT - 1:
                # ---- normalize pair (c, p) ----
                last = (c == NCH - 1 and p == 1)
                base = 32 * c + 16 * p
                if last:
                    # DMA-free normalize: recip row + PE broadcast + mul,
                    # den/cast copies on the now-idle scalar engine. PE
                    # keepalive: K=1 matmuls data-dep-pinned to each chain
                    # stage (Tile would reschedule dependency-free matmuls
                    # anywhere) so the PE never stalls >3.4us (HAM).
                    def wpad(rhs_row):
                        warmt = fil_pool.tile([64, QC], F32, tag="fil",
                                              name="warmt")
                        nc.tensor.matmul(warmt[:, :], lhsT=wsrc[0:1, 0:64],
                                         rhs=rhs_row, start=True, stop=True)

                    for h01 in range(2):
                        aug = augs[h01]
                        den0 = rec_pool.tile([1, QC], F32, tag="den0",
                                             name="den0")
                        nc.scalar.copy(out=den0[0:1, :], in_=aug[64:65, :])
                        cpa = cpa_pool.tile([65, QC], F32, tag="cpa",
                                            name="cpa")
                        nc.vector.tensor_copy(out=cpa[:, :], in_=aug[:, :])
                        wpad(den0.bitcast(BF16)[0:1, 0:QC])
                        rec = rec_pool.tile([1, QC], F32, tag="rec", name="rec")
                        nc.vector.reciprocal_approx_fast(out=rec[0:1, :],
                                                         in_=den0[0:1, :])
                        recb = rec_pool.tile([1, QC], BF16, tag="recb",
                                             name="recb")
                        nc.scalar.copy(out=recb[0:1, :], in_=rec[0:1, :])
                        wpad(recb[0:1, :])
                        bc = fil_pool.tile([64, QC], F32, tag="fil", name="bc")
                        nc.tensor.matmul(bc[:, :], lhsT=wsrc[0:1, 0:64],
                                         rhs=recb[0:1, :],
                                         start=True, stop=True)
                        nc.vector.tensor_mul(
                            out=outT_sb[p][64 * h01:64 * h01 + 64,
                                           c * QC:(c + 1) * QC],
                            in0=cpa[0:64, :],
                            in1=bc[:, :],
                        )
                else:
                    # steady-state normalize: zero PE, minimal DVE — denom
                    # spread + reciprocal 16-wide, DRAM-bounce broadcast; the
                    # ~7us DMA latency hides behind the exp stream. DMA issue
                    # on sync/gpsimd queues; final mul on the GPSIMD engine
                    # (all-SBUF operands) to keep DVE free for Schraudolph.
                    for h01 in range(2):
                        aug = augs[h01]
                        cpa = cpa_pool.tile([65, QC], F32, tag="cpa",
                                            name="cpa")
                        nc.vector.tensor_copy(out=cpa[:, :], in_=aug[:, :])
                        d16 = rec_pool.tile([16, 32], F32, tag="d16",
                                            name="d16")
                        nc.sync.dma_start(out=d16[:, :], in_=cpa[64:65, :])
                        rec = rec_pool.tile([16, 32], F32, tag="rec16",
                                            name="rec")
                        nc.vector.reciprocal(out=rec[:, :], in_=d16[:, :])
                        rec_dr = dram_pool.tile([1, QC], F32, tag="recd",
                                                name="rec_dr")
                        nc.sync.dma_start(out=rec_dr[:, :], in_=rec[:, :])
                        bcs = cpa_pool.tile([64, QC], F32, tag="bcs",
                                            name="bcs")
                        nc.gpsimd.dma_start(
                            out=bcs[:, :],
                            in_=rec_dr[0:1, :].to_broadcast([64, QC]))

                        def bc_mul(p=p, c=c, h01=h01, cpa=cpa, bcs=bcs):
                            nc.gpsimd.tensor_mul(
                                out=outT_sb[p][64 * h01:64 * h01 + 64,
                                               c * QC:(c + 1) * QC],
                                in0=cpa[0:64, :],
                                in1=bcs[:, :],
                            )

                        fillers.append((base + 20 + h01, bc_mul))
                    fillers.sort(key=lambda u: u[0])
                if p == 1:
                    if c < NCH - 1:
                        for ot in range(4):
                            fillers.append((32 * (c + 1) + 6 + 6 * ot,
                                            lambda ot=ot, c=c: proj_unit(ot, c)))
                        fillers.sort(key=lambda u: u[0])
                    else:
                        # tail: last chunk's projection; bias adds on the
                        # now-idle scalar engine, stores on sync+scalar queues
                        for ot in range(4):
                            proj_unit(ot, c, tail=True)


def build_nc():
    nc = bacc.Bacc("TRN2", target_bir_lowering=False, debug=False, num_devices=8)
    xT = nc.dram_tensor("xT", [P, 4 * N], BF16, kind="ExternalInput").ap()
    wqk = nc.dram_tensor("wqk", [P, 4 * 512], BF16, kind="ExternalInput").ap()
    wv = nc.dram_tensor("wv", [P, 4 * CG], BF16, kind="ExternalInput").ap()
    wpT = nc.dram_tensor("wpT", [P, 2 * C], BF16, kind="ExternalInput").ap()
    bias = nc.dram_tensor("bias", [C], F32, kind="ExternalInput").ap()
    yT = nc.dram_tensor("yT", [P, 4 * 2048], F16, kind="ExternalOutput").ap()
    _build_body(nc, xT, wqk, wv, wpT, bias, yT)
    nc.compile()
    return nc


def make_in_maps(x, w_qkv, w_proj, b_proj):
    in_maps = []
    for core in range(8):
        b, g = core // 2, core % 2
        q01 = np.arange(CG * g, CG * g + P)
        q23 = np.arange(CG * g + P, CG * g + 2 * P)
        k01 = C + q01
        k23 = C + q23
        vrows = np.arange(2 * C + CG * g, 2 * C + CG * (g + 1))
        wqk = np.concatenate([w_qkv[q01], w_qkv[k01], w_qkv[q23], w_qkv[k23]])
        # xT host: [p, q*2048 + ct*512 + j] = x.T[ct*128+p, q*512+j]
        xt = x[b].T.reshape(4, P, 4, QC).transpose(1, 2, 0, 3).reshape(P, 4 * N)
        # [p, jt*512 + ct*128]: column blocks by output tile then ct
        wqkh = (wqk.T.reshape(4, P, 4, P).transpose(1, 2, 0, 3)
                .reshape(P, 2048))
        wvh = w_qkv[vrows].T.reshape(4, P, CG).transpose(1, 0, 2).reshape(P, 4 * CG)
        wph = (w_proj[:, CG * g:CG * (g + 1)].T
               .reshape(2, P, C).transpose(1, 0, 2).reshape(P, 2 * C))
        in_maps.append({
            "xT": np.ascontiguousarray(xt).astype(ml_dtypes.bfloat16),
            "wqk": np.ascontiguousarray(wqkh).astype(ml_dtypes.bfloat16),
            "wv": np.ascontiguousarray(wvh).astype(ml_dtypes.bfloat16),
            "wpT": np.ascontiguousarray(wph).astype(ml_dtypes.bfloat16),
            "bias": (b_proj if g == 0 else np.zeros_like(b_proj)).astype(np.float32),
        })
    return in_maps


_NC = None


def _get_nc():
    global _NC
    if _NC is None:
        _NC = build_nc()
    return _NC


def run(x, w_qkv, w_proj, b_proj, trace=False, **kw):
    nc = _get_nc()
    in_maps = make_in_maps(
        np.asarray(x), np.asarray(w_qkv), np.asarray(w_proj), np.asarray(b_proj)
    )
    res = run_bass_kernel_spmd(nc, in_maps, list(range(8)), trace=trace, **kw)
    out = np.empty((B, N, C), np.float32)
    for b in range(B):
        yv = (res.results[2 * b]["yT"].astype(np.float32)
              + res.results[2 * b + 1]["yT"].astype(np.float32))
        # stage col = c*2048 + ot*512 + j ; y channel = ot*128+p, token c*512+j
        out[b] = (yv.reshape(P, 4, 4, QC).transpose(2, 0, 1, 3)
                  .reshape(C, N)).T
    return out, res


def kernel(x, w_qkv, w_proj, b_proj):
    out, _ = run(x, w_qkv, w_proj, b_proj, trace=False)
    return out


# revision 15
# speedup vs baseline: 1.0252x; 1.0252x over previous
"""Trainium2 Bass kernel for nn_Attention_7421703487529.

Multi-head attention, B=4 N=2048 C=512 H=8 D=64, fp32.
Sharding: 8 cores = 4 batches x 2 head-groups (4 heads each). No collectives.

Design (all per core; 128 iterations over (chunk c:4, pair p:2, key-tile i:16)):
- The wall is exp throughput: 4 heads x 2048^2 softmax exps. Split across TWO
  engines: ScalarE runs [128,1024] exp ACTIVATEs (~1.11us each, covers both
  heads of a pair for a 512-query chunk); DVE runs Schraudolph exp for tiles
  with i%2==0, i>0, m>=16 (49 of 128): one tensor_scalar P_int =
  int32(s*(2^23*log2 e*SCALE) + B) whose int32 bits ARE the fp32 exp approx;
  the AV matmul reads the high bf16 halves via a stride-2 AP. B is centered
  (mean multiplicative bias ~1) and +0x8000 rounds-to-nearest the bf16 half.
- AV-deferral: for DVE-offloaded tiles the AV pair is deferred one iteration
  (emitted after the NEXT S-pair) so a late DVE tensor_scalar can never stall
  the PE before S(k+3) is out -> the ScalarE exp stream never starves while
  skipping offloaded tiles.
- Score matmuls (K=64) for the two heads run CONCURRENTLY via PE row-tiling
  into one [128,1024] PSUM tile; S(k+2) is emitted before AV(k).
- V is computed directly in [token, dim] layout into 66-wide slots with a
  ones column at index 64 -> the AV matmul accumulates numerator rows 0-63 +
  denominator row 64.
- Steady normalize: aug->SBUF copy (DVE), denominator spread via SBUF-SBUF DMA
  (sync queue), 16-wide reciprocal (DVE), DRAM-bounce broadcast (gpsimd
  queue), final mul on the GPSIMD engine (all-SBUF operands). The bc+mul is
  deferred via the filler queue so a late broadcast can't stall the PE.
- Projection: psum + bias staged as ONE [128,2048] f16 tile per chunk via DVE
  tensor_scalar add (bias as per-partition scalar AP), stored with a single
  128-descriptor DMA on the sync queue (4x fewer store descriptors).
- Tail (last pair): DMA-free normalize chain with PE-keepalive K=1 matmuls
  data-dep-pinned to the chain (a >3.4us PE stall trips HAM re-throttle to
  1.2 GHz); projections' bias-adds on the then-idle ScalarE; final store as
  2 half DMAs on sync+scalar queues.
- Inputs land as big DMAs from host-interleaved [128, x] layouts; ~104 tiny
  warm-up matmuls bridge the DMA wait to keep the PE HAM clock at 2.4 GHz.
"""

import sys

for _p in ("/opt/trn_rl_repo", "/root/.axon_site/_ro/trn_rl_repo"):
    if _p not in sys.path:
        sys.path.append(_p)

import ml_dtypes
import numpy as np

import concourse.bass as bass
import concourse.tile as tile
from concourse import bacc, mybir
from concourse.bass_utils import run_bass_kernel_spmd

F32 = mybir.dt.float32
F16 = mybir.dt.float16
BF16 = mybir.dt.bfloat16

B, N, C = 4, 2048, 512
H, D = 8, 64
HG = 2              # head-groups (cores per batch)
HL = H // HG        # heads per core (4)
CG = C // HG        # channels per group (256)
SCALE = D ** -0.5
P = 128             # partitions
NT = N // P         # 16 key tiles per head
NCH = 4             # query chunks
QC = N // NCH       # 512 queries per chunk
SLOT = 66           # vall slot width: 64 V dims + ones col (64) + pad
EXP = mybir.ActivationFunctionType.Exp


def _build_body(nc, xT, wqk, wv, wpT, bias, yT):
    from contextlib import ExitStack

    with tile.TileContext(nc) as tc, ExitStack() as ctx:
        consts = ctx.enter_context(tc.tile_pool(name="consts", bufs=1))
        pT_pool = ctx.enter_context(tc.tile_pool(name="pT", bufs=8))
        cpa_pool = ctx.enter_context(tc.tile_pool(name="cpa", bufs=3))
        rec_pool = ctx.enter_context(tc.tile_pool(name="rec", bufs=3))
        stg_pool = ctx.enter_context(tc.tile_pool(name="stg", bufs=2))
        pS_pool = ctx.enter_context(tc.tile_pool(name="pS", bufs=5))
        dram_pool = ctx.enter_context(tc.tile_pool(name="drp", bufs=2, space="DRAM"))
        # per-head score tiles: releasing ssA (after the head-A half-exp or
        # half-Schraudolph) unblocks the next S_A matmul while head B is
        # still being consumed -> the ss-buf recycle chain (exp(k-1) ->
        # S(k+1) -> exp(k+1)) hides behind the half-exp instead of opening
        # a ~0.6us ScalarE hole at every offloaded tile
        ssA_pool = ctx.enter_context(tc.tile_pool(name="ssA", bufs=2, space="PSUM"))
        ssB_pool = ctx.enter_context(tc.tile_pool(name="ssB", bufs=2, space="PSUM"))
        aug_pool = ctx.enter_context(tc.tile_pool(name="aug", bufs=2, space="PSUM"))
        fil_pool = ctx.enter_context(tc.tile_pool(name="fil", bufs=2, space="PSUM"))

        # ---- HAM warm-up: tiny back-to-back matmuls promote the PE clock to
        # 2.4 GHz while input DMAs are in flight (no DMA dependency) ----
        wsrc = consts.tile([64, 64], BF16, tag="wsrc")
        nc.vector.memset(wsrc.bitcast(mybir.dt.uint16), 0x3F80)
        warm = fil_pool.tile([64, 64], F32, tag="fil", name="warm")
        for _ in range(104):
            nc.tensor.matmul(warm[:, :], lhsT=wsrc[:, :], rhs=wsrc[:, :],
                             start=True, stop=True)

        # ---- input loads: few big DMAs from host-interleaved layouts ----
        # xT dram: [128, chunk q (4) x ct (4) x 512]; wqk: [128, ct x 512]
        # (ct-blocks each [Q01|K01|Q23|K23]); wv: [128, ct x 256]; wp:
        # [128, ct2 x 512]
        xT_all = consts.tile([P, 4 * N], BF16, tag="xT_all")
        wqk_all = consts.tile([P, 4 * 512], BF16, tag="wqk_all")
        wv_all = consts.tile([P, 4 * CG], BF16, tag="wv_all")
        wp_all = consts.tile([P, 2 * C], BF16, tag="wp_all")
        bias_sb = consts.tile([P, 4], F32, tag="bias")

        # critical pair for the first S tiles: one wide DMA each (descriptor
        # count is per partition row, so width is free) on two queues; the
        # remaining loads spread across the three DMA-capable queues
        nc.sync.dma_start(out=wqk_all[:, 0:1024], in_=wqk[:, 0:1024])
        nc.gpsimd.dma_start(out=xT_all[:, 0:2048], in_=xT[:, 0:2048])
        nc.scalar.dma_start(out=wv_all, in_=wv[:, :])
        nc.scalar.dma_start(out=xT_all[:, 2048:4096], in_=xT[:, 2048:4096])
        nc.sync.dma_start(out=wqk_all[:, 1024:2048], in_=wqk[:, 1024:2048])
        nc.gpsimd.dma_start(out=xT_all[:, 4096:6144], in_=xT[:, 4096:6144])
        nc.sync.dma_start(out=xT_all[:, 6144:8192], in_=xT[:, 6144:8192])
        nc.gpsimd.dma_start(out=wp_all, in_=wpT[:, :])
        nc.scalar.dma_start(
            out=bias_sb,
            in_=bass.AP(tensor=bias.tensor, offset=bias.offset, ap=[[1, P], [P, 4]]),
        )
        # trigger the exp ACT-table load (~2.7us) before the first real exp;
        # placed after the scalar-queue DMA issues so those aren't delayed
        actwarm = consts.tile([1, 1], F32, tag="actwarm")
        nc.scalar.activation(out=actwarm[0:1, :],
                             in_=nc.const_aps.aps[(F32, 1.0)][0:1, :],
                             func=EXP, scale=1.0)

        def x_sl(ct, q, j0, w):
            return xT_all[:, q * 2048 + ct * QC + j0: q * 2048 + ct * QC + j0 + w]

        # ---- SBUF working tensors ----
        # qkvT tiles: 0=Q01, 1=K01, 2=Q23, 3=K23 ([128, N]: pair head A rows
        # 0-63, head B rows 64-127)
        qkvT_sb = [consts.tile([P, N], BF16, tag=f"qkvT{jt}", name=f"qkvT{jt}")
                   for jt in range(4)]
        # V slots: [p, i, l, c]: c=0:64 V dims, c=64 ones, c=65 pad
        vall = consts.tile([P, NT * HL * SLOT], BF16, tag="vall")
        vall4 = vall.rearrange("p (i l c) -> p i l c", l=HL, c=SLOT)
        nc.vector.memset(vall4[:, :, :, 64].bitcast(mybir.dt.uint16), 0x3F80)
        nc.vector.memset(vall4[:, :, :, 65].bitcast(mybir.dt.uint16), 0)
        outT_sb = [consts.tile([P, N], BF16, tag=f"outT{t}", name=f"outT{t}")
                   for t in range(2)]
        ones_col = consts.tile([P, 64], BF16, tag="ones_col")
        nc.vector.memset(ones_col.bitcast(mybir.dt.uint16), 0x3F80)

        # ---- emission helpers ----
        def qkv_quarter(jt, q):
            # qkvT_sb[jt][:, q*512:(q+1)*512] = wqk[:, jt].T @ x[:, qchunk]
            ps = fil_pool.tile([P, QC], F32, tag="fil", name="ps_qkv")
            for ct in range(4):
                nc.tensor.matmul(
                    ps[:, :],
                    lhsT=wqk_all[:, jt * 512 + ct * P:jt * 512 + (ct + 1) * P],
                    rhs=x_sl(ct, q, 0, QC),
                    start=(ct == 0),
                    stop=(ct == 3),
                )
            nc.vector.tensor_copy(out=qkvT_sb[jt][:, q * QC:(q + 1) * QC],
                                  in_=ps[:, :])

        def v_unit(i):
            # V[tokens 128i.., dims 256] = x_chunk @ wv ; scatter to head slots
            ps = fil_pool.tile([P, CG], F32, tag="fil", name="ps_v")
            for ct in range(4):
                nc.tensor.matmul(
                    ps[:, :],
                    lhsT=x_sl(ct, i // 4, (i % 4) * P, P),
                    rhs=wv_all[:, ct * CG:(ct + 1) * CG],
                    start=(ct == 0),
                    stop=(ct == 3),
                )
            ps3 = ps.rearrange("p (l c) -> p l c", l=HL)
            nc.vector.tensor_copy(out=vall4[:, i, :, 0:64], in_=ps3[:, :, :])

        # f16 output staging: one [128, 4*QC] tile per chunk; col ot*QC+j
        stages = {}

        def proj_unit(ot, c, tail=False):
            # y[ot rows, chunk c] = wp.T-slices @ outT + bias -> f16 staging
            ps = fil_pool.tile([P, QC], F32, tag="fil", name="ps_y")
            for ct in range(2):
                nc.tensor.matmul(
                    ps[:, :],
                    lhsT=wp_all[:, ct * C + ot * P:ct * C + (ot + 1) * P],
                    rhs=outT_sb[ct][:, c * QC:(c + 1) * QC],
                    start=(ct == 0),
                    stop=(ct == 1),
                )
            if ot == 0:
                stages[c] = stg_pool.tile([P, 4 * QC], F16, tag="stg",
                                          name=f"stg{c}")
            stg = stages[c]
            if tail:
                nc.scalar.add(out=stg[:, ot * QC:(ot + 1) * QC], in_=ps[:, :],
                              add=bias_sb[:, ot:ot + 1])
            else:
                nc.vector.tensor_scalar(
                    out=stg[:, ot * QC:(ot + 1) * QC], in0=ps[:, :],
                    scalar1=bias_sb[:, ot:ot + 1], scalar2=None,
                    op0=mybir.AluOpType.add)
            if ot == 3:
                if tail:
                    nc.sync.dma_start(out=yT[:, c * 2048:c * 2048 + 1024],
                                      in_=stg[:, 0:1024])
                    nc.scalar.dma_start(out=yT[:, c * 2048 + 1024:(c + 1) * 2048],
                                        in_=stg[:, 1024:2048])
                else:
                    nc.sync.dma_start(out=yT[:, c * 2048:(c + 1) * 2048],
                                      in_=stg[:, :])

        # ---- prologue compute: ONLY what exp#0 needs (cold clock) ----
        qkv_quarter(0, 0)      # Q01 cols 0:512
        qkv_quarter(1, 0)      # K01 cols 0:512 (key tiles 0-3)

        # filler queue: (deadline iteration, fn). 128 iterations total;
        # iteration index = 32*c + 16*p + i. qkv quarters are due >=4 iters
        # before first use so their DVE copies clear the boundary DVE bursts.
        fillers = []

        def q(dl, fn, *a):
            fillers.append((dl, lambda: fn(*a)))

        for i in range(NT):
            q(i - 1, v_unit, i)                     # V token tiles (JIT)
        q(0, qkv_quarter, 1, 1)                     # K01 tiles 4-7
        q(4, qkv_quarter, 1, 2)
        q(8, qkv_quarter, 1, 3)
        q(11, qkv_quarter, 2, 0)                    # Q23 cols 0:512
        q(12, qkv_quarter, 3, 0)                    # K23 tiles 0-3
        q(14, qkv_quarter, 3, 1)
        q(16, qkv_quarter, 3, 2)
        q(20, qkv_quarter, 3, 3)
        q(24, qkv_quarter, 0, 1)                    # Q01 chunk 1
        q(40, qkv_quarter, 2, 1)                    # Q23 chunk 1
        q(56, qkv_quarter, 0, 2)
        q(72, qkv_quarter, 2, 2)
        q(88, qkv_quarter, 0, 3)
        q(104, qkv_quarter, 2, 3)

        # tail PE keepalive: the S stream ends ~iteration 125 while ScalarE
        # still drains exps; a PE stall >3.4us trips the HAM re-throttle to
        # 1.2 GHz right before the tail. Dummy 512-col matmuls keep it fed.
        def keepalive():
            # reads the chunk-3 pair-0 outT region (written ~iteration 118)
            # so Tile cannot hoist these dependency-free matmuls earlier
            warmk = fil_pool.tile([64, QC], F32, tag="fil", name="warmk")
            nc.tensor.matmul(warmk[:, :], lhsT=wsrc[0:1, 0:64],
                             rhs=outT_sb[0][0:1, 3 * QC:4 * QC],
                             start=True, stop=True)

        for dl in (121, 122, 123, 124, 125, 126, 127):
            q(dl, keepalive)
        fillers.sort(key=lambda u: u[0])

        # ---- attention: 128 iterations of (chunk, pair, key-tile) ----
        blocks = [(c, p) for c in range(NCH) for p in range(2)]
        iters = [(c, p, i) for (c, p) in blocks for i in range(NT)]

        def emit_S(k):
            c, p, i = iters[k]
            QT, KT = qkvT_sb[2 * p], qkvT_sb[2 * p + 1]
            ssA = ssA_pool.tile([P, QC], F32, tag="ssA", name="ssA")
            ssB = ssB_pool.tile([P, QC], F32, tag="ssB", name="ssB")
            nc.tensor.matmul(
                ssA[:, :],
                lhsT=KT[0:64, i * P:(i + 1) * P],
                rhs=QT[0:64, c * QC:(c + 1) * QC],
                start=True, stop=True,
            )
            nc.tensor.matmul(
                ssB[:, :],
                lhsT=KT[64:128, i * P:(i + 1) * P],
                rhs=QT[64:128, c * QC:(c + 1) * QC],
                start=True, stop=True,
            )
            return (ssA, ssB)

        # Schraudolph exp on DVE for offloaded tiles: P = bitcast(int32(
        # s*A + B)); +0x8000 rounds-to-nearest the bf16 high half the AV
        # matmul reads via a stride-2 AP. Any per-tile scalar bias cancels
        # between softmax numerator and denominator.
        SCHRAU_A = float(SCALE * (2 ** 23) / np.log(2.0))
        SCHRAU_B = float(127 * 2 ** 23 - 482870 + 0x8000)
        I32 = mybir.dt.int32

        def offloaded(m):
            ci, pi, ii = iters[m]
            return ii % 2 == 0 and ii > 0 and m >= 16

        def emit_tile(m):
            # S-pair matmuls; for DVE-offloaded tiles the Schraudolph
            # tensor_scalars are emitted HERE (2 iterations ahead in the DVE
            # queue) so they start as soon as the scores land; per-head ops
            # release ssA early for the next S_A
            ssab = emit_S(m)
            if offloaded(m):
                pS = pS_pool.tile([P, 2 * QC], I32, tag="pS", name="pS")
                for h in range(2):
                    nc.vector.tensor_scalar(out=pS[:, h * QC:(h + 1) * QC],
                                            in0=ssab[h][:, :],
                                            scalar1=SCHRAU_A, scalar2=SCHRAU_B,
                                            op0=mybir.AluOpType.mult,
                                            op1=mybir.AluOpType.add)
                return (ssab, [
                    pS[:, h * QC:(h + 1) * QC].bitcast(BF16)
                    .rearrange("p (n two) -> p n two", two=2)[:, :, 1]
                    for h in range(2)
                ])
            return (ssab, None)

        def emit_AV(augs_, p_, i_, rhs_):
            for h01 in range(2):
                nc.tensor.matmul(
                    augs_[h01][:, :],
                    lhsT=vall4[:, i_, 2 * p_ + h01, 0:65],
                    rhs=rhs_[h01],
                    start=(i_ == 0),
                    stop=(i_ == NT - 1),
                )

        ss_q = [emit_tile(0), emit_tile(1)]
        augs = None
        av_q = []            # deferred (augs, p, i, rhs_ab) AV pairs
        done = 0
        for k, (c, p, i) in enumerate(iters):
            if i == 0:
                augs = (aug_pool.tile([65, QC], F32, tag="aug", name="augA"),
                        aug_pool.tile([65, QC], F32, tag="aug", name="augB"))
            ssab, pre_rhs = ss_q.pop(0)
            if pre_rhs is not None:
                rhs_ab = pre_rhs
            else:
                pT = pT_pool.tile([P, 2 * QC], BF16, tag="pT")
                for h in range(2):
                    nc.scalar.activation(out=pT[:, h * QC:(h + 1) * QC],
                                         in_=ssab[h][:, :], func=EXP,
                                         scale=float(SCALE))
                rhs_ab = [pT[:, 0:QC], pT[:, QC:2 * QC]]
            # flush deferred AV pairs in 2-iteration batches: their inputs
            # are 1-2 slots old -> wait-free, and 4 consecutive same-config
            # AV matmuls pipeline back-to-back (fewer S<->AV array-config
            # switches); also hides the ss-recycle chain behind real work
            if len(av_q) >= 2:
                for a_ in av_q:
                    emit_AV(*a_)
                av_q.clear()
            if k + 2 < len(iters):
                ss_q.append(emit_tile(k + 2))
            # fillers: force-pop overdue units (correctness: a unit's write
            # must be EMITTED before its first reader), then a bounded
            # near-deadline drain
            npop = 0
            while fillers and fillers[0][0] <= done:
                fillers.pop(0)[1]()
                npop += 1
            while fillers and npop < 2 and (fillers[0][0] <= done + 3
                                            or (done % 3 == 0 and npop == 0)):
                fillers.pop(0)[1]()
                npop += 1
            done += 1

            if i < NT - 1:
                # defer the AV pair (i=15 must stay in its own pair: the aug
                # pool has no room to let accumulation cross a pair boundary)
                av_q.append((augs, p, i, rhs_ab))
            else:
                for a_ in av_q:
                    emit_AV(*a_)
                av_q.clear()
                emit_AV(augs, p, i, rhs_ab)
            if i == NT - 1:
                # ---- normalize pair (c, p) ----
                last = (c == NCH - 1 and p == 1)
                base = 32 * c + 16 * p
                if last:
                    # DMA-free normalize: recip row + PE broadcast + mul,
                    # den/cast copies on the now-idle scalar engine. PE
                    # keepalive: K=1 matmuls data-dep-pinned to each chain
                    # stage (Tile would reschedule dependency-free matmuls
                    # anywhere) so the PE never stalls >3.4us (HAM).
                    def wpad(rhs_row):
                        warmt = fil_pool.tile([64, QC], F32, tag="fil",
                                              name="warmt")
                        nc.tensor.matmul(warmt[:, :], lhsT=wsrc[0:1, 0:64],
                                         rhs=rhs_row, start=True, stop=True)

                    for h01 in range(2):
                        aug = augs[h01]
                        den0 = rec_pool.tile([1, QC], F32, tag="den0",
                                             name="den0")
                        nc.scalar.copy(out=den0[0:1, :], in_=aug[64:65, :])
                        cpa = cpa_pool.tile([65, QC], F32, tag="cpa",
                                            name="cpa")
                        nc.vector.tensor_copy(out=cpa[:, :], in_=aug[:, :])
                        wpad(den0.bitcast(BF16)[0:1, 0:QC])
                        rec = rec_pool.tile([1, QC], F32, tag="rec", name="rec")
                        nc.vector.reciprocal_approx_fast(out=rec[0:1, :],
                                                         in_=den0[0:1, :])
                        recb = rec_pool.tile([1, QC], BF16, tag="recb",
                                             name="recb")
                        nc.scalar.copy(out=recb[0:1, :], in_=rec[0:1, :])
                        wpad(recb[0:1, :])
                        bc = fil_pool.tile([64, QC], F32, tag="fil", name="bc")
                        nc.tensor.matmul(bc[:, :], lhsT=wsrc[0:1, 0:64],
                                         rhs=recb[0:1, :],
                                         start=True, stop=True)
                        nc.vector.tensor_mul(
                            out=outT_sb[p][64 * h01:64 * h01 + 64,
                                           c * QC:(c + 1) * QC],
                            in0=cpa[0:64, :],
                            in1=bc[:, :],
                        )
                else:
                    # steady-state normalize: zero PE, minimal DVE — denom
                    # spread + reciprocal 16-wide, DRAM-bounce broadcast; the
                    # ~7us DMA latency hides behind the exp stream. DMA issue
                    # on sync/gpsimd queues; final mul on the GPSIMD engine
                    # (all-SBUF operands) to keep DVE free for Schraudolph.
                    for h01 in range(2):
                        aug = augs[h01]
                        cpa = cpa_pool.tile([65, QC], F32, tag="cpa",
                                            name="cpa")
                        nc.vector.tensor_copy(out=cpa[:, :], in_=aug[:, :])
                        d16 = rec_pool.tile([16, 32], F32, tag="d16",
                                            name="d16")
                        nc.sync.dma_start(out=d16[:, :], in_=cpa[64:65, :])
                        rec = rec_pool.tile([16, 32], F32, tag="rec16",
                                            name="rec")
                        nc.vector.reciprocal(out=rec[:, :], in_=d16[:, :])
                        rec_dr = dram_pool.tile([1, QC], F32, tag="recd",
                                                name="rec_dr")
                        nc.sync.dma_start(out=rec_dr[:, :], in_=rec[:, :])
                        bcs = cpa_pool.tile([64, QC], F32, tag="bcs",
                                            name="bcs")
                        nc.gpsimd.dma_start(
                            out=bcs[:, :],
                            in_=rec_dr[0:1, :].to_broadcast([64, QC]))

                        def bc_mul(p=p, c=c, h01=h01, cpa=cpa, bcs=bcs):
                            nc.gpsimd.tensor_mul(
                                out=outT_sb[p][64 * h01:64 * h01 + 64,
                                               c * QC:(c + 1) * QC],
                                in0=cpa[0:64, :],
                                in1=bcs[:, :],
                            )

                        fillers.append((base + 20 + h01, bc_mul))
                    fillers.sort(key=lambda u: u[0])
                if p == 1:
                    if c < NCH - 1:
                        for ot in range(4):
                            fillers.append((32 * (c + 1) + 6 + 6 * ot,
                                            lambda ot=ot, c=c: proj_unit(ot, c)))
                        fillers.sort(key=lambda u: u[0])
                    else:
                        # tail: last chunk's projection; bias adds on the
                        # now-idle scalar engine, stores on sync+scalar queues
                        for ot in range(4):
                            proj_unit(ot, c, tail=True)


def build_nc():
    nc = bacc.Bacc("TRN2", target_bir_lowering=False, debug=False, num_devices=8)
    xT = nc.dram_tensor("xT", [P, 4 * N], BF16, kind="ExternalInput").ap()
    wqk = nc.dram_tensor("wqk", [P, 4 * 512], BF16, kind="ExternalInput").ap()
    wv = nc.dram_tensor("wv", [P, 4 * CG], BF16, kind="ExternalInput").ap()
    wpT = nc.dram_tensor("wpT", [P, 2 * C], BF16, kind="ExternalInput").ap()
    bias = nc.dram_tensor("bias", [C], F32, kind="ExternalInput").ap()
    yT = nc.dram_tensor("yT", [P, 4 * 2048], F16, kind="ExternalOutput").ap()
    _build_body(nc, xT, wqk, wv, wpT, bias, yT)
    nc.compile()
    return nc


def make_in_maps(x, w_qkv, w_proj, b_proj):
    in_maps = []
    for core in range(8):
        b, g = core // 2, core % 2
        q01 = np.arange(CG * g, CG * g + P)
        q23 = np.arange(CG * g + P, CG * g + 2 * P)
        k01 = C + q01
        k23 = C + q23
        vrows = np.arange(2 * C + CG * g, 2 * C + CG * (g + 1))
        wqk = np.concatenate([w_qkv[q01], w_qkv[k01], w_qkv[q23], w_qkv[k23]])
        # xT host: [p, q*2048 + ct*512 + j] = x.T[ct*128+p, q*512+j]
        xt = x[b].T.reshape(4, P, 4, QC).transpose(1, 2, 0, 3).reshape(P, 4 * N)
        # [p, jt*512 + ct*128]: column blocks by output tile then ct
        wqkh = (wqk.T.reshape(4, P, 4, P).transpose(1, 2, 0, 3)
                .reshape(P, 2048))
        wvh = w_qkv[vrows].T.reshape(4, P, CG).transpose(1, 0, 2).reshape(P, 4 * CG)
        wph = (w_proj[:, CG * g:CG * (g + 1)].T
               .reshape(2, P, C).transpose(1, 0, 2).reshape(P, 2 * C))
        in_maps.append({
            "xT": np.ascontiguousarray(xt).astype(ml_dtypes.bfloat16),
            "wqk": np.ascontiguousarray(wqkh).astype(ml_dtypes.bfloat16),
            "wv": np.ascontiguousarray(wvh).astype(ml_dtypes.bfloat16),
            "wpT": np.ascontiguousarray(wph).astype(ml_dtypes.bfloat16),
            "bias": (b_proj if g == 0 else np.zeros_like(b_proj)).astype(np.float32),
        })
    return in_maps


_NC = None


def _get_nc():
    global _NC
    if _NC is None:
        _NC = build_nc()
    return _NC


def run(x, w_qkv, w_proj, b_proj, trace=False, **kw):
    nc = _get_nc()
    in_maps = make_in_maps(
        np.asarray(x), np.asarray(w_qkv), np.asarray(w_proj), np.asarray(b_proj)
    )
    res = run_bass_kernel_spmd(nc, in_maps, list(range(8)), trace=trace, **kw)
    out = np.empty((B, N, C), np.float32)
    for b in range(B):
        yv = (res.results[2 * b]["yT"].astype(np.float32)
              + res.results[2 * b + 1]["yT"].astype(np.float32))
        # stage col = c*2048 + ot*512 + j ; y channel = ot*128+p, token c*512+j
        out[b] = (yv.reshape(P, 4, 4, QC).transpose(2, 0, 1, 3)
                  .reshape(C, N)).T
    return out, res


def kernel(x, w_qkv, w_proj, b_proj):
    out, _ = run(x, w_qkv, w_proj, b_proj, trace=False)
    return out
